# revision 23
# baseline (speedup 1.0000x reference)
"""Trainium2 Bass kernel for a Swin-style transformer block
(windowed attention with RoPE + SwiGLU MLP with sub-LN).

Sharding: data-parallel over batch B=8 -> one image per NeuronCore.

v2 scheme (vs v1 baseline):
- fp8(e4m3) + DoubleRow matmuls for q/k/v/proj GEMMs (2x PE throughput);
  attention probs in fp8(e5m2) with a constant -5 exp shift (cancels in
  softmax), AV matmul in DoubleRow over the two 128/68 key chunks with
  zero-padded V rows annihilating pad/fake keys.
- MLP (w1/w2/w3) stays bf16: fp8 there fails the 2e-2 gate (measured
  3.3e-2 host-side).
- LN1/LN2 stats via fp32r ones-matmuls on x / x^2 (replicated psum out,
  no gpsimd all-reduce); rstd via sqrt(Act) + fast-reciprocal(DVE).
- hid-LN mean folded into w3 via an extra (-colsum(w3), mean) rank-1
  matmul row; A applied on the w3 output.
- Scores for both key chunks land in one PSUM bank -> single Exp per
  window-head; single DoubleRow AV per window-head; per-head-pair z
  reciprocal + broadcast-DMA + one normalize mult.
- Elementwise work split across DVE / Act / Pool (engine map below).

Numerics vs fp32 reference (host emulation): ~3.5e-3 absmax-rel.
"""
import numpy as np
import ml_dtypes
from contextlib import ExitStack

import concourse.bass as bass
import concourse.tile as tile
from concourse import bacc, mybir
from concourse.bass_utils import run_bass_kernel_spmd

BF16NP = ml_dtypes.bfloat16
F8E4NP = ml_dtypes.float8_e4m3
F8E5NP = ml_dtypes.float8_e5m2
F32 = mybir.dt.float32
F32R = mybir.dt.float32r
BF16 = mybir.dt.bfloat16
F8E4 = mybir.dt.float8e4
F8E5 = mybir.dt.float8e5
OP = mybir.AluOpType
AF = mybir.ActivationFunctionType
DRM = mybir.MatmulPerfMode.DoubleRow

DIM = 768
HEADS = 12
HD = 64
HID = 2048
EPS = 1e-6
WS = 14
NTOK = WS * WS          # 196
B, H, W = 8, 64, 64
NWIN = 25
TOKS = NWIN * NTOK      # 4900
KT = DIM // 128         # 6
MT = HID // 128         # 16
N_CORES = 8
P = 128
PC2 = 2 * NTOK          # 392
KPAD = 64               # khat column pad so chunk-1 lhsT always in-bounds
SA = 16.0               # h1 fp8 scale
SV = 16.0               # v / ohat fp8 scale
ESH = -5.0              # exp shift (cancels in softmax)

USE_SILU = True          # CoreSim lacks Silu; simtest flips this off
USE_RECIP_FAST = True    # fall back to nc.vector.reciprocal if False

_cache = {}


def _rope_tables():
    dim, pt, theta = 32, 16.0, 10000.0
    freqs = 1.0 / theta ** (np.arange(0, dim, 2, dtype=np.float32) / dim)
    f1 = np.repeat((np.arange(WS, dtype=np.float32) / WS * pt)[:, None] * freqs[None, :], 2, axis=-1)
    f = np.concatenate([
        np.broadcast_to(f1[:, None, :], (WS, WS, dim)),
        np.broadcast_to(f1[None, :, :], (WS, WS, dim)),
    ], -1).reshape(NTOK, 2 * dim)
    return np.cos(f), np.sin(f)   # [196, 64]


# head-pair banks for AV: (h, h+2) share a psum bank, adjacent k-tiles
AV_PAIRS = [(0, 2), (1, 3), (4, 6), (5, 7), (8, 10), (9, 11)]


def _emit(nc, tc, ctx, aps, hb, sc, nwin_total=NWIN, loop_n=1, phase='full'):
    pairs = []
    w = 0
    while w < nwin_total:
        pairs.append((w, w + 1) if w + 1 < nwin_total else (w,))
        w += 2

    xT = aps["xT"].rearrange("(k p) n -> p k n", p=P)
    yT = aps["yT"].rearrange("(k p) n -> p k n", p=P)

    consts = ctx.enter_context(tc.tile_pool(name="consts", bufs=1))
    wpool = ctx.enter_context(tc.tile_pool(name="weights", bufs=1))
    xpool = ctx.enter_context(tc.tile_pool(name="x", bufs=2))
    sqpool = ctx.enter_context(tc.tile_pool(name="xsq", bufs=1))
    lnpool = ctx.enter_context(tc.tile_pool(name="ln", bufs=1))
    nrmpool = ctx.enter_context(tc.tile_pool(name="nrm", bufs=3))
    hpool = ctx.enter_context(tc.tile_pool(name="h", bufs=1))
    qspool = ctx.enter_context(tc.tile_pool(name="qs", bufs=2))
    qkpool = ctx.enter_context(tc.tile_pool(name="qk", bufs=1))
    vpool = ctx.enter_context(tc.tile_pool(name="v", bufs=1))
    epool = ctx.enter_context(tc.tile_pool(name="e", bufs=3))
    zpool = ctx.enter_context(tc.tile_pool(name="z", bufs=3))
    opool = ctx.enter_context(tc.tile_pool(name="ohat", bufs=1))
    x1pool = ctx.enter_context(tc.tile_pool(name="x1", bufs=1))
    mlppool = ctx.enter_context(tc.tile_pool(name="mlp", bufs=2))
    gpool = ctx.enter_context(tc.tile_pool(name="g", bufs=1))
    ypool = ctx.enter_context(tc.tile_pool(name="y", bufs=2))

    ps_mm = ctx.enter_context(tc.tile_pool(name="psmm", bufs=2, space="PSUM"))
    ps_sc = ctx.enter_context(tc.tile_pool(name="pssc", bufs=2, space="PSUM"))
    ps_av = ctx.enter_context(tc.tile_pool(name="psav", bufs=2, space="PSUM"))
    ps_st = ctx.enter_context(tc.tile_pool(name="psst", bufs=1, space="PSUM"))

    # ---- weights ----
    def load_w(name, kdim, mdim, dt):
        t = wpool.tile([P, kdim // P, mdim], dt, tag=name)
        nc.sync.dma_start(t[:], aps[name].rearrange("(k p) m -> p k m", p=P))
        return t

    wq = load_w("wq", DIM, DIM, F8E4)
    wk = load_w("wk", DIM, DIM, F8E4)
    wv = load_w("wv", DIM, DIM, F8E4)
    wp = load_w("wp", DIM, DIM, F8E4)
    w1 = load_w("w1", DIM, HID, BF16)
    w2 = load_w("w2", DIM, HID, BF16)
    w3 = load_w("w3", HID, DIM, BF16)

    cos2 = consts.tile([P, PC2], BF16, tag="cos2")
    nc.sync.dma_start(cos2[:], aps["cos2"][:])
    sin2 = consts.tile([P, PC2], BF16, tag="sin2")
    nc.sync.dma_start(sin2[:], aps["sin2"][:])
    r2t = consts.tile([P, P], BF16, tag="r2t")
    nc.sync.dma_start(r2t[:], aps["r2t"][:])
    w3csr = consts.tile([1, DIM], BF16, tag="w3csr")   # -colsum(w3)
    nc.sync.dma_start(w3csr[:], aps["w3csr"][:])

    ones_b = consts.tile([P, P], BF16, tag="ones_b")
    nc.vector.memset(ones_b[:], 1.0)
    onesrow = consts.tile([1, PC2], BF16, tag="onesrow")
    nc.vector.memset(onesrow[:], 1.0)
    eps1 = consts.tile([P, 1], F32, tag="eps1")
    nc.vector.memset(eps1[:], float(DIM) * float(DIM) * EPS)
    eps3 = consts.tile([P, 1], F32, tag="eps3")
    nc.vector.memset(eps3[:], float(HID) * float(HID) * EPS)
    eshc = consts.tile([P, 1], F32, tag="eshc")
    nc.vector.memset(eshc[:], ESH)

    def bias_row(name, feat):
        if aps.get(name) is None:
            return None
        t = consts.tile([1, feat], BF16, tag=name)
        nc.sync.dma_start(t[:], aps[name][:])
        return t

    qb = bias_row("qb", DIM)      # pre-scaled on host to psum units
    kb = bias_row("kb", DIM)
    vb = bias_row("vb", DIM)
    pb = bias_row("pb", DIM)
    w1b = bias_row("w1b", HID)
    w2b = bias_row("w2b", HID)
    w3b = aps.get("w3b") is not None
    w3bc = None
    if w3b:
        w3bc = consts.tile([P, KT], F32, tag="w3bc")
        nc.sync.dma_start(w3bc[:], aps["w3b"].rearrange("(k p) -> p k", p=P))

    def emit_pair(wins):
        nwin = len(wins)
        pc = NTOK * nwin
        c0 = wins[0] * NTOK
        chunks = [(0, 196)] if nwin == 1 else [(0, 196), (196, 196)]

        x_t = xpool.tile([P, KT, PC2], F32, tag="x")
        nc.sync.dma_start(x_t[:, :, :pc], xT[:, :, c0:c0 + pc])

        # ---------- LN stats: bf16 casts + ones^T matmuls (replicated) ----------
        def ln_stats(src):
            xb = sqpool.tile([P, KT, PC2], BF16, tag="xb")
            xsq = sqpool.tile([P, KT, PC2], BF16, tag="xsq")
            for k in range(KT):
                nc.gpsimd.tensor_scalar_add(out=xb[:, k, :pc], in0=src[:, k, :pc],
                                            scalar1=0.0)
                nc.scalar.activation(out=xsq[:, k, :pc], in_=src[:, k, :pc],
                                     func=AF.Square, bias=0.0, scale=1.0)
            S = ps_st.tile([P, PC2], F32, tag="S")
            Q = ps_st.tile([P, PC2], F32, tag="Q")
            for k in range(KT):
                nc.tensor.matmul(S[:, :pc], lhsT=ones_b[:], rhs=xb[:, k, :pc],
                                 start=(k == 0), stop=(k == KT - 1),
                                 skip_group_check=True)
            for k in range(KT):
                nc.tensor.matmul(Q[:, :pc], lhsT=ones_b[:], rhs=xsq[:, k, :pc],
                                 start=(k == 0), stop=(k == KT - 1),
                                 skip_group_check=True)
            return S, Q

        def ln_tail(S, Q, n, epscol, ab_scale, tag, want_cb=True):
            # rstd = n / sqrt(n*Q - S^2 + n^2 eps); Ab = ab_scale * rstd
            # tags shared across the three LNs (sequential use) to save SBUF
            tag = ""
            t0 = lnpool.tile([P, PC2], F32, tag=tag + "t0")
            nc.scalar.activation(out=t0[:, :pc], in_=S[:, :pc],
                                 func=AF.Square, bias=0.0, scale=1.0)
            nc.vector.scalar_tensor_tensor(out=t0[:, :pc], in0=Q[:, :pc], scalar=float(n),
                                           in1=t0[:, :pc], op0=OP.mult, op1=OP.subtract)
            nc.scalar.activation(out=t0[:, :pc], in_=t0[:, :pc], func=AF.Sqrt,
                                 bias=epscol[:], scale=1.0)
            if USE_RECIP_FAST:
                nc.vector.reciprocal_approx_fast(out=t0[:, :pc], in_=t0[:, :pc])
            else:
                nc.vector.reciprocal(out=t0[:, :pc], in_=t0[:, :pc])
            Ab = lnpool.tile([P, PC2], F32, tag=tag + "ab")
            nc.vector.tensor_scalar_mul(out=Ab[:, :pc], in0=t0[:, :pc],
                                        scalar1=float(n) * ab_scale)
            if not want_cb:
                return Ab, None
            Cb = lnpool.tile([P, PC2], F32, tag=tag + "cb")
            nc.vector.tensor_scalar_mul(out=Cb[:, :pc], in0=S[:, :pc],
                                        scalar1=1.0 / float(n))
            return Ab, Cb

        def normalize(src, Ab, Cb, dst, dt):
            for k in range(KT):
                tmp = nrmpool.tile([P, PC2], F32, tag="nrm")
                nc.vector.tensor_tensor(out=tmp[:, :pc], in0=src[:, k, :pc],
                                        in1=Cb[:, :pc], op=OP.subtract)
                nc.gpsimd.tensor_tensor(out=dst[:, k, :pc], in0=tmp[:, :pc],
                                        in1=Ab[:, :pc], op=OP.mult)

        if phase == "dma":
            return
        S1, Q1 = ln_stats(x_t)
        Ab1, Cb1 = ln_tail(S1, Q1, DIM, eps1, SA, "l1")
        h1 = hpool.tile([P, KT, 512], F8E4, tag="h1")
        nc.gpsimd.memset(h1[:, :, pc:], 0.0)
        normalize(x_t, Ab1, Cb1, h1, F8E4)

        if phase == "ln":
            return
        # ---------- q/k projections (fp8 DoubleRow) + RoPE ----------
        def dr_group(psum, wmat, rhs_t, m, extra_bias=None, cols=None):
            """Accumulate wmat[:, :, m*128:(m+1)*128]^T @ rhs over K in one
            psum bank using DoubleRow; col-chunks share the group via the
            overwrite-where-unset semantics of start=False."""
            cols = chunks if cols is None else cols
            nkp = wmat.shape[1] // 2
            last = (len(cols) - 1, nkp - 1)
            for ci, (cc0, ccn) in enumerate(cols):
                for kp in range(nkp):
                    st = (kp == 0)
                    sp = (ci, kp) == last and extra_bias is None
                    nc.tensor.matmul(psum[:, cc0:cc0 + ccn],
                                     lhsT=wmat[:, 2 * kp:2 * kp + 2, m * P:(m + 1) * P],
                                     rhs=rhs_t[:, 2 * kp:2 * kp + 2, cc0:cc0 + ccn],
                                     start=st, stop=sp, perf_mode=DRM,
                                     skip_group_check=True)
            if extra_bias is not None:
                nc.tensor.matmul(psum[:, :pc], lhsT=extra_bias[:, m * P:(m + 1) * P],
                                 rhs=onesrow[:, :pc], start=False, stop=True,
                                 skip_group_check=True)

        def emit_qk(wmat, brow, dest, ds):
            for m in range(KT):
                ps = ps_mm.tile([P, PC2], F32, tag="mm")
                dr_group(ps, wmat, h1, m, extra_bias=brow)
                qs = qspool.tile([P, PC2], BF16, tag="qs")
                nc.scalar.activation(out=qs[:, :pc], in_=ps[:, :pc],
                                     func=AF.Copy, bias=0.0, scale=ds)
                rot = ps_mm.tile([P, PC2], F32, tag="mm")
                nc.tensor.matmul(rot[:, :pc], lhsT=r2t[:], rhs=qs[:, :pc],
                                 start=True, stop=True)
                t1 = qspool.tile([P, PC2], BF16, tag="t1")
                nc.vector.tensor_tensor(out=t1[:, :pc], in0=qs[:, :pc],
                                        in1=cos2[:, :pc], op=OP.mult)
                t2 = qspool.tile([P, PC2], BF16, tag="t2")
                nc.vector.tensor_tensor(out=t2[:, :pc], in0=rot[:, :pc],
                                        in1=sin2[:, :pc], op=OP.mult)
                nc.gpsimd.tensor_tensor(out=dest[:, m, :pc], in0=t1[:, :pc],
                                        in1=t2[:, :pc], op=OP.add)

        qhat = qkpool.tile([P, KT, PC2], BF16, tag="qhat")
        khat = qkpool.tile([P, KT, PC2 + KPAD], BF16, tag="khat")
        nc.gpsimd.memset(khat[:, :, pc:], 0.0)
        emit_qk(wq, qb, qhat, sc["ds_q"])
        emit_qk(wk, kb, khat, sc["ds_k"])

        if phase == "qk":
            return
        # ---------- V (token-major, fp8 DoubleRow) ----------
        v_ts = []
        for wi in range(nwin):
            wcol = wi * NTOK
            v_t = vpool.tile([P, HEADS, 2, HD + 1], BF16, tag=f"v{wi}")
            nc.gpsimd.memset(v_t[64:128, :, 1, :], 0.0)
            for ci, (cs, cn) in enumerate([(0, 128), (128, 68)]):
                for half in range(2):
                    ps = ps_mm.tile([P, PC2], F32, tag="mm")
                    for q4 in range(2):
                        for kp in range(3):
                            st = (kp == 0)
                            sp = (q4 == 1 and kp == 2) and vb is None
                            nc.tensor.matmul(
                                ps[:, q4 * 192:(q4 + 1) * 192],
                                lhsT=h1[:, 2 * kp:2 * kp + 2, wcol + cs:wcol + cs + 128],
                                rhs=wv[:, 2 * kp:2 * kp + 2,
                                       half * 384 + q4 * 192:half * 384 + (q4 + 1) * 192],
                                start=st, stop=sp, perf_mode=DRM,
                                skip_group_check=True)
                    if vb is not None:
                        nc.tensor.matmul(ps[:, 0:384], lhsT=onesrow[:, 0:128],
                                         rhs=vb[:, half * 384:(half + 1) * 384],
                                         start=False, stop=True, skip_group_check=True)
                    nc.scalar.activation(
                        out=v_t[0:cn, half * 6:(half + 1) * 6, ci, 0:HD],
                        in_=ps[0:cn, 0:384].rearrange("p (h d) -> p h d", d=HD),
                        func=AF.Copy, bias=0.0, scale=sc["ds_v"])
            nc.gpsimd.memset(v_t[:, :, 0, HD:HD + 1], 0.0625)
            nc.vector.memset(v_t[0:68, :, 1, HD:HD + 1], 0.0625)
            v_ts.append(v_t)

        if phase == "v":
            return
        # ---------- attention ----------
        ohat = None
        if phase not in ("att1", "att2", "att3a", "att3b"):
            ohat = opool.tile([P, KT, PC2], F8E4, tag="ohat")
        for wi in range(nwin):
            wcol = wi * NTOK
            v_t = v_ts[wi]

            def head_exp(hh):
                r0 = 64 * (hh % 2)
                g6 = hh // 2
                sps = ps_sc.tile([P, PC2], F32, tag="sc")
                nc.tensor.matmul(sps[:, 0:196],
                                 lhsT=khat[r0:r0 + 64, g6, wcol:wcol + 128],
                                 rhs=qhat[r0:r0 + 64, g6, wcol:wcol + NTOK],
                                 start=True, stop=True, skip_group_check=True)
                nc.tensor.matmul(sps[:, 196:392],
                                 lhsT=khat[r0:r0 + 64, g6, wcol + 128:wcol + 256],
                                 rhs=qhat[r0:r0 + 64, g6, wcol:wcol + NTOK],
                                 start=True, stop=True, skip_group_check=True)
                e2 = epool.tile([P, 2, NTOK], BF16, tag="e2")
                nc.scalar.activation(out=e2[:].rearrange("p a b -> p (a b)"),
                                     in_=sps[:, :], func=AF.Exp, bias=eshc[:], scale=1.0)
                return e2

            for (ha, hc) in AV_PAIRS:
                ea = head_exp(ha)
                ec = head_exp(hc)
                if phase == "att1":
                    continue
                avp = ps_av.tile([P, PC2], F32, tag="av")
                for si, (hh, ee) in enumerate(((ha, ea), (hc, ec))):
                    nc.tensor.matmul(avp[0:HD + 1, si * 196:(si + 1) * 196],
                                     lhsT=v_t[:, hh, 0, :], rhs=ee[:, 0, :],
                                     start=True, stop=False, skip_group_check=True)
                    nc.tensor.matmul(avp[0:HD + 1, si * 196:(si + 1) * 196],
                                     lhsT=v_t[:, hh, 1, :], rhs=ee[:, 1, :],
                                     start=False, stop=True, skip_group_check=True)
                if phase == "att2":
                    continue
                zc = zpool.tile([1, PC2], F32, tag="zc")
                nc.vector.reciprocal(out=zc[:], in_=avp[HD:HD + 1, :])
                if phase == "att3a":
                    continue
                zb = zpool.tile([64, PC2], F32, tag="zb")
                zap = zc[:]
                nc.sync.dma_start(zb[:], bass.AP(tensor=zap.tensor, offset=zap.offset,
                                                 ap=[zap.ap[0], [0, 64], zap.ap[1]]))
                if phase == "att3b":
                    continue
                r0 = 64 * (ha % 2)
                g6 = ha // 2
                nc.vector.tensor_tensor(
                    out=ohat[r0:r0 + 64, g6:g6 + 2, wcol:wcol + NTOK],
                    in0=avp[0:64, :].rearrange("p (a b) -> p a b", a=2),
                    in1=zb[:].rearrange("p (a b) -> p a b", a=2),
                    op=OP.mult)

        if phase in ("att", "att1", "att2", "att3a", "att3b"):
            return
        # ---------- proj + residual ----------
        x1 = x1pool.tile([P, KT, PC2], F32, tag="x1")
        for m in range(KT):
            ps = ps_mm.tile([P, PC2], F32, tag="mm")
            dr_group(ps, wp, ohat, m, extra_bias=pb)
            nc.vector.scalar_tensor_tensor(out=x1[:, m, :pc], in0=ps[:, :pc],
                                           scalar=sc["ds_p"], in1=x_t[:, m, :pc],
                                           op0=OP.mult, op1=OP.add)

        if phase == "proj":
            return
        # ---------- LN2 + MLP ----------
        S2, Q2 = ln_stats(x1)
        Ab2, Cb2 = ln_tail(S2, Q2, DIM, eps1, 1.0, "l2")
        h2 = hpool.tile([P, KT, PC2], BF16, tag="h2")
        normalize(x1, Ab2, Cb2, h2, BF16)

        g = gpool.tile([P, MT, PC2], BF16, tag="g")
        sg = ps_st.tile([P, PC2], F32, tag="S")
        qg = ps_st.tile([P, PC2], F32, tag="Q")
        for m in range(MT):
            p1 = ps_mm.tile([P, PC2], F32, tag="mm")
            for k in range(KT):
                nc.tensor.matmul(p1[:, :pc], lhsT=w1[:, k, m * P:(m + 1) * P],
                                 rhs=h2[:, k, :pc], start=(k == 0),
                                 stop=(k == KT - 1 and w1b is None))
            if w1b is not None:
                nc.tensor.matmul(p1[:, :pc], lhsT=w1b[:, m * P:(m + 1) * P],
                                 rhs=onesrow[:, :pc], start=False, stop=True,
                                 skip_group_check=True)
            sf = mlppool.tile([P, PC2], BF16, tag="sf")
            if USE_SILU:
                nc.scalar.activation(out=sf[:, :pc], in_=p1[:, :pc], func=AF.Silu,
                                     bias=0.0, scale=1.0)
            else:
                s1 = mlppool.tile([P, PC2], BF16, tag="s1")
                nc.scalar.activation(out=s1[:, :pc], in_=p1[:, :pc], func=AF.Sigmoid,
                                     bias=0.0, scale=1.0)
                nc.vector.tensor_tensor(out=sf[:, :pc], in0=p1[:, :pc],
                                        in1=s1[:, :pc], op=OP.mult)
            p2 = ps_mm.tile([P, PC2], F32, tag="mm")
            for k in range(KT):
                nc.tensor.matmul(p2[:, :pc], lhsT=w2[:, k, m * P:(m + 1) * P],
                                 rhs=h2[:, k, :pc], start=(k == 0),
                                 stop=(k == KT - 1 and w2b is None))
            if w2b is not None:
                nc.tensor.matmul(p2[:, :pc], lhsT=w2b[:, m * P:(m + 1) * P],
                                 rhs=onesrow[:, :pc], start=False, stop=True,
                                 skip_group_check=True)
            nc.vector.tensor_tensor(out=g[:, m, :pc], in0=p2[:, :pc],
                                    in1=sf[:, :pc], op=OP.mult)
            sqg = mlppool.tile([P, PC2], BF16, tag="sqg")
            nc.vector.tensor_tensor(out=sqg[:, :pc], in0=g[:, m, :pc],
                                    in1=g[:, m, :pc], op=OP.mult)
            nc.tensor.matmul(sg[:, :pc], lhsT=ones_b[:], rhs=g[:, m, :pc],
                             start=(m == 0), stop=(m == MT - 1), skip_group_check=True)
            nc.tensor.matmul(qg[:, :pc], lhsT=ones_b[:], rhs=sqg[:, :pc],
                             start=(m == 0), stop=(m == MT - 1), skip_group_check=True)

        if phase == "mlp":
            return
        At, _ = ln_tail(sg, qg, HID, eps3, 1.0, "l3", want_cb=False)
        mrow = lnpool.tile([1, PC2], BF16, tag="mrow")
        nc.vector.tensor_scalar_mul(out=mrow[:, :pc], in0=sg[0:1, :pc],
                                    scalar1=1.0 / float(HID))

        # ---------- w3 + hid-LN scale + residual ----------
        for m in range(KT):
            ps = ps_mm.tile([P, PC2], F32, tag="mm")
            for k in range(MT):
                nc.tensor.matmul(ps[:, :pc], lhsT=w3[:, k, m * P:(m + 1) * P],
                                 rhs=g[:, k, :pc], start=(k == 0), stop=False,
                                 skip_group_check=True)
            nc.tensor.matmul(ps[:, :pc], lhsT=w3csr[:, m * P:(m + 1) * P],
                             rhs=mrow[:, :pc], start=False, stop=True,
                             skip_group_check=True)
            yt = ypool.tile([P, PC2], F32, tag="yt")
            nc.vector.tensor_tensor(out=yt[:, :pc], in0=ps[:, :pc],
                                    in1=At[:, :pc], op=OP.mult)
            if w3bc is not None:
                nc.vector.scalar_tensor_tensor(out=yt[:, :pc], in0=yt[:, :pc],
                                               scalar=w3bc[:, m:m + 1],
                                               in1=x1[:, m, :pc], op0=OP.add, op1=OP.add)
            else:
                nc.vector.tensor_tensor(out=yt[:, :pc], in0=yt[:, :pc],
                                        in1=x1[:, m, :pc], op=OP.add)
            nc.sync.dma_start(yT[:, m, c0:c0 + pc], yt[:, :pc])

    def emit_all():
        for wins in pairs:
            emit_pair(wins)
        if phase != "full":
            yt0 = ypool.tile([P, PC2], F32, tag="yt")
            nc.vector.memset(yt0[:], 0.0)
            nc.sync.dma_start(yT[:, 0, 0:PC2], yt0[:])

    if loop_n > 1:
        with tc.For_i(0, loop_n, 1):
            emit_all()
    else:
        emit_all()


def _build(has_biases, nwin_total=NWIN, ncores=N_CORES, loop_n=1, scales=None, phase='full'):
    key = ("prog", USE_SILU, USE_RECIP_FAST, tuple(sorted(has_biases.items())), nwin_total, ncores, loop_n,
           tuple(sorted((scales or {}).items())), phase)
    if key in _cache:
        return _cache[key]
    nc = bacc.Bacc("TRN2", target_bir_lowering=False, debug=False,
                   enable_asserts=False, num_devices=ncores)
    toks = nwin_total * NTOK
    aps = {}
    aps["xT"] = nc.dram_tensor("xT", [DIM, toks], F32, kind="ExternalInput").ap()
    aps["yT"] = nc.dram_tensor("yT", [DIM, toks], F32, kind="ExternalOutput").ap()
    for nm, shp, dt in [("wq", [DIM, DIM], F8E4), ("wk", [DIM, DIM], F8E4),
                        ("wv", [DIM, DIM], F8E4), ("wp", [DIM, DIM], F8E4),
                        ("w1", [DIM, HID], BF16), ("w2", [DIM, HID], BF16),
                        ("w3", [HID, DIM], BF16)]:
        aps[nm] = nc.dram_tensor(nm, shp, dt, kind="ExternalInput").ap()
    aps["cos2"] = nc.dram_tensor("cos2", [P, PC2], BF16, kind="ExternalInput").ap()
    aps["sin2"] = nc.dram_tensor("sin2", [P, PC2], BF16, kind="ExternalInput").ap()
    aps["r2t"] = nc.dram_tensor("r2t", [P, P], BF16, kind="ExternalInput").ap()
    aps["w3csr"] = nc.dram_tensor("w3csr", [1, DIM], BF16, kind="ExternalInput").ap()
    for nm, d in [("qb", DIM), ("kb", DIM), ("vb", DIM), ("pb", DIM),
                  ("w1b", HID), ("w2b", HID)]:
        aps[nm] = (nc.dram_tensor(nm, [1, d], BF16, kind="ExternalInput").ap()
                   if has_biases.get(nm) else None)
    aps["w3b"] = (nc.dram_tensor("w3b", [DIM], F32, kind="ExternalInput").ap()
                  if has_biases.get("w3b") else None)
    sc = scales or {"ds_q": 1.0, "ds_k": 1.0, "ds_v": 1.0, "ds_p": 1.0}
    with tile.TileContext(nc) as tc:
        with ExitStack() as ctx:
            _emit(nc, tc, ctx, aps, has_biases, sc, nwin_total, loop_n, phase)
    nc.compile()
    _cache[key] = nc
    return nc


def _pick_scale(w, target=160.0):
    a = float(np.abs(w).max())
    return 2.0 ** np.floor(np.log2(target / a)) if a > 0 else 1.0


def _host_prep(inputs):
    f = {k: np.asarray(v, np.float32) if hasattr(v, "shape") else v
         for k, v in inputs.items()}
    scale = HD ** -0.5
    wq = f["ln1_w"][:, None] * f["q_w"] * scale
    wk = f["ln1_w"][:, None] * f["k_w"]
    wv = f["ln1_w"][:, None] * f["v_w"]
    qb = (f["ln1_b"] @ f["q_w"] + f["q_b"]) * scale
    kb = f["ln1_b"] @ f["k_w"]
    vb = f["ln1_b"] @ f["v_w"] + f["v_b"]
    wp = f["proj_w"]
    pb = f["proj_b"]
    w1 = f["ln2_w"][:, None] * f["w1_w"]
    w2 = f["ln2_w"][:, None] * f["w2_w"]
    w1b = f["ln2_b"] @ f["w1_w"] + f["w1_b"]
    w2b = f["ln2_b"] @ f["w2_w"] + f["w2_b"]
    w3 = f["ffn_w"][:, None] * f["w3_w"]
    w3b = f["ffn_b"] @ f["w3_w"] + f["w3_b"]

    sq, sk, sv, sp = (_pick_scale(wq), _pick_scale(wk),
                      _pick_scale(wv), _pick_scale(wp))
    scales = {"ds_q": 1.0 / (sq * SA), "ds_k": 1.0 / (sk * SA),
              "ds_v": 1.0 / (sv * SA), "ds_p": 1.0 / (sp * SV)}

    cos, sin = _rope_tables()
    cosT = np.ascontiguousarray(cos.T)
    sinT = np.ascontiguousarray(sin.T)
    cos2 = np.tile(np.concatenate([cosT, cosT], 0), (1, 2))
    sin2 = np.tile(np.concatenate([sinT, sinT], 0), (1, 2))

    r = np.zeros((64, 64), np.float32)
    for i in range(32):
        r[2 * i, 2 * i + 1] = -1.0
        r[2 * i + 1, 2 * i] = 1.0
    r2 = np.zeros((128, 128), np.float32)
    r2[:64, :64] = r
    r2[64:, 64:] = r
    r2t = np.ascontiguousarray(r2.T)

    x = f["x"]
    pad = (-H) % WS
    nw = (H + pad) // WS
    xp = np.pad(x, ((0, 0), (0, pad), (0, pad), (0, 0)))
    t = xp.reshape(B, nw, WS, nw, WS, DIM).transpose(0, 1, 3, 2, 4, 5).reshape(B, NWIN * NTOK, DIM)

    shared = {
        "wq": (wq * sq).astype(F8E4NP), "wk": (wk * sk).astype(F8E4NP),
        "wv": (wv * sv).astype(F8E4NP), "wp": (wp * sp).astype(F8E4NP),
        "w1": w1.astype(BF16NP), "w2": w2.astype(BF16NP), "w3": w3.astype(BF16NP),
        "cos2": cos2.astype(BF16NP), "sin2": sin2.astype(BF16NP),
        "r2t": r2t.astype(BF16NP),
        "w3csr": np.ascontiguousarray(-w3.sum(0)[None, :]).astype(BF16NP),
    }
    # biases pre-scaled into psum units of their group
    brows = {"qb": qb * (sq * SA), "kb": kb * (sk * SA), "vb": vb * (sv * SA),
             "pb": pb * (sp * SV), "w1b": w1b, "w2b": w2b}
    has_biases = {k: bool(np.any(np.asarray(v) != 0.0)) for k, v in brows.items()}
    has_biases["w3b"] = bool(np.any(w3b != 0.0))
    for k, v in brows.items():
        if has_biases[k]:
            shared[k] = np.ascontiguousarray(v[None, :]).astype(BF16NP)
    if has_biases["w3b"]:
        shared["w3b"] = np.ascontiguousarray(w3b, np.float32)

    in_maps = []
    for b in range(B):
        m = dict(shared)
        m["xT"] = np.ascontiguousarray(t[b].T)
        in_maps.append(m)
    return in_maps, has_biases, scales


def _host_post(results):
    pad = (-H) % WS
    nw = (H + pad) // WS
    Hp = H + pad
    y = np.empty((B, H, W, DIM), np.float32)
    for b in range(B):
        yb = np.asarray(results[b]["yT"])
        yw = yb.T.reshape(nw, nw, WS, WS, DIM).transpose(0, 2, 1, 3, 4).reshape(Hp, Hp, DIM)
        y[b] = yw[:H, :W, :]
    return y


def kernel(**inputs):
    in_maps, has_biases, scales = _host_prep(inputs)
    nc = _build(has_biases, scales=scales)
    res = run_bass_kernel_spmd(nc, in_maps, core_ids=list(range(N_CORES)))
    return _host_post(res.results)


# revision 24
# speedup vs baseline: 1.1724x; 1.1724x over previous
"""Trainium2 Bass kernel for a Swin-style transformer block
(windowed attention with RoPE + SwiGLU MLP with sub-LN).

Sharding: data-parallel over batch B=8 -> one image per NeuronCore.

v2 scheme (vs v1 baseline):
- fp8(e4m3) + DoubleRow matmuls for q/k/v/proj GEMMs (2x PE throughput);
  attention probs in fp8(e5m2) with a constant -5 exp shift (cancels in
  softmax), AV matmul in DoubleRow over the two 128/68 key chunks with
  zero-padded V rows annihilating pad/fake keys.
- MLP (w1/w2/w3) stays bf16: fp8 there fails the 2e-2 gate (measured
  3.3e-2 host-side).
- LN1/LN2 stats via fp32r ones-matmuls on x / x^2 (replicated psum out,
  no gpsimd all-reduce); rstd via sqrt(Act) + fast-reciprocal(DVE).
- hid-LN mean folded into w3 via an extra (-colsum(w3), mean) rank-1
  matmul row; A applied on the w3 output.
- Scores for both key chunks land in one PSUM bank -> single Exp per
  window-head; single DoubleRow AV per window-head; per-head-pair z
  reciprocal + broadcast-DMA + one normalize mult.
- Elementwise work split across DVE / Act / Pool (engine map below).

Numerics vs fp32 reference (host emulation): ~3.5e-3 absmax-rel.
"""
import numpy as np
import ml_dtypes
from contextlib import ExitStack

import concourse.bass as bass
import concourse.tile as tile
from concourse import bacc, mybir
from concourse.bass_utils import run_bass_kernel_spmd

BF16NP = ml_dtypes.bfloat16
F8E4NP = ml_dtypes.float8_e4m3
F8E5NP = ml_dtypes.float8_e5m2
F32 = mybir.dt.float32
F32R = mybir.dt.float32r
BF16 = mybir.dt.bfloat16
F8E4 = mybir.dt.float8e4
F8E5 = mybir.dt.float8e5
OP = mybir.AluOpType
AF = mybir.ActivationFunctionType
DRM = mybir.MatmulPerfMode.DoubleRow

DIM = 768
HEADS = 12
HD = 64
HID = 2048
EPS = 1e-6
WS = 14
NTOK = WS * WS          # 196
B, H, W = 8, 64, 64
NWIN = 25
TOKS = NWIN * NTOK      # 4900
KT = DIM // 128         # 6
MT = HID // 128         # 16
N_CORES = 8
P = 128
PC2 = 2 * NTOK          # 392
KPAD = 64               # khat column pad so chunk-1 lhsT always in-bounds
SA = 16.0               # h1 fp8 scale
SV = 16.0               # v / ohat fp8 scale
ESH = -5.0              # exp shift (cancels in softmax)

USE_SILU = True          # CoreSim lacks Silu; simtest flips this off
USE_RECIP_FAST = True    # fall back to nc.vector.reciprocal if False

_cache = {}


def _rope_tables():
    dim, pt, theta = 32, 16.0, 10000.0
    freqs = 1.0 / theta ** (np.arange(0, dim, 2, dtype=np.float32) / dim)
    f1 = np.repeat((np.arange(WS, dtype=np.float32) / WS * pt)[:, None] * freqs[None, :], 2, axis=-1)
    f = np.concatenate([
        np.broadcast_to(f1[:, None, :], (WS, WS, dim)),
        np.broadcast_to(f1[None, :, :], (WS, WS, dim)),
    ], -1).reshape(NTOK, 2 * dim)
    return np.cos(f), np.sin(f)   # [196, 64]


# head-pair banks for AV: (h, h+2) share a psum bank, adjacent k-tiles
AV_PAIRS = [(0, 2), (1, 3), (4, 6), (5, 7), (8, 10), (9, 11)]


def _emit(nc, tc, ctx, aps, hb, sc, nwin_total=NWIN, loop_n=1, phase='full'):
    pairs = []
    w = 0
    while w < nwin_total:
        pairs.append((w, w + 1) if w + 1 < nwin_total else (w,))
        w += 2

    xT = aps["xT"].rearrange("(k p) n -> p k n", p=P)
    yT = aps["yT"].rearrange("(k p) n -> p k n", p=P)

    consts = ctx.enter_context(tc.tile_pool(name="consts", bufs=1))
    wpool = ctx.enter_context(tc.tile_pool(name="weights", bufs=1))
    xpool = ctx.enter_context(tc.tile_pool(name="x", bufs=2))
    sqpool = ctx.enter_context(tc.tile_pool(name="xsq", bufs=1))
    lnpool = ctx.enter_context(tc.tile_pool(name="ln", bufs=1))
    nrmpool = ctx.enter_context(tc.tile_pool(name="nrm", bufs=3))
    hpool = ctx.enter_context(tc.tile_pool(name="h", bufs=1))
    qspool = ctx.enter_context(tc.tile_pool(name="qs", bufs=2))
    qkpool = ctx.enter_context(tc.tile_pool(name="qk", bufs=1))
    vpool = ctx.enter_context(tc.tile_pool(name="v", bufs=1))
    epool = ctx.enter_context(tc.tile_pool(name="e", bufs=3))
    zpool = ctx.enter_context(tc.tile_pool(name="z", bufs=3))
    opool = ctx.enter_context(tc.tile_pool(name="ohat", bufs=1))
    x1pool = ctx.enter_context(tc.tile_pool(name="x1", bufs=1))
    mlppool = ctx.enter_context(tc.tile_pool(name="mlp", bufs=2))
    gpool = ctx.enter_context(tc.tile_pool(name="g", bufs=1))
    ypool = ctx.enter_context(tc.tile_pool(name="y", bufs=2))

    ps_mm = ctx.enter_context(tc.tile_pool(name="psmm", bufs=2, space="PSUM"))
    ps_sc = ctx.enter_context(tc.tile_pool(name="pssc", bufs=2, space="PSUM"))
    ps_av = ctx.enter_context(tc.tile_pool(name="psav", bufs=2, space="PSUM"))
    ps_st = ctx.enter_context(tc.tile_pool(name="psst", bufs=1, space="PSUM"))

    # ---- weights ----
    def load_w(name, kdim, mdim, dt):
        t = wpool.tile([P, kdim // P, mdim], dt, tag=name)
        nc.sync.dma_start(t[:], aps[name].rearrange("(k p) m -> p k m", p=P))
        return t

    wq = load_w("wq", DIM, DIM, F8E4)
    wk = load_w("wk", DIM, DIM, F8E4)
    wv = load_w("wv", DIM, DIM, F8E4)
    wp = load_w("wp", DIM, DIM, F8E4)
    w1 = load_w("w1", DIM, HID, BF16)
    w2 = load_w("w2", DIM, HID, BF16)
    w3 = load_w("w3", HID, DIM, BF16)

    cos2 = consts.tile([P, PC2], BF16, tag="cos2")
    nc.sync.dma_start(cos2[:], aps["cos2"][:])
    sin2 = consts.tile([P, PC2], BF16, tag="sin2")
    nc.sync.dma_start(sin2[:], aps["sin2"][:])
    r2t = consts.tile([P, P], BF16, tag="r2t")
    nc.sync.dma_start(r2t[:], aps["r2t"][:])
    w3csr = consts.tile([1, DIM], BF16, tag="w3csr")   # -colsum(w3)
    nc.sync.dma_start(w3csr[:], aps["w3csr"][:])

    ones_b = consts.tile([P, P], BF16, tag="ones_b")
    nc.vector.memset(ones_b[:], 1.0)
    onesrow = consts.tile([1, PC2], BF16, tag="onesrow")
    nc.vector.memset(onesrow[:], 1.0)
    eps1 = consts.tile([P, 1], F32, tag="eps1")
    nc.vector.memset(eps1[:], float(DIM) * float(DIM) * EPS)
    eps3 = consts.tile([P, 1], F32, tag="eps3")
    nc.vector.memset(eps3[:], float(HID) * float(HID) * EPS)
    eshc = consts.tile([P, 1], F32, tag="eshc")
    nc.vector.memset(eshc[:], ESH)

    def bias_row(name, feat):
        if aps.get(name) is None:
            return None
        t = consts.tile([1, feat], BF16, tag=name)
        nc.sync.dma_start(t[:], aps[name][:])
        return t

    qb = bias_row("qb", DIM)      # pre-scaled on host to psum units
    kb = bias_row("kb", DIM)
    vb = bias_row("vb", DIM)
    pb = bias_row("pb", DIM)
    w1b = bias_row("w1b", HID)
    w2b = bias_row("w2b", HID)
    w3b = aps.get("w3b") is not None
    w3bc = None
    if w3b:
        w3bc = consts.tile([P, KT], F32, tag="w3bc")
        nc.sync.dma_start(w3bc[:], aps["w3b"].rearrange("(k p) -> p k", p=P))

    def emit_pair(wins):
        nwin = len(wins)
        pc = NTOK * nwin
        c0 = wins[0] * NTOK
        chunks = [(0, 196)] if nwin == 1 else [(0, 196), (196, 196)]

        x_t = xpool.tile([P, KT, PC2], F32, tag="x")
        nc.sync.dma_start(x_t[:, :, :pc], xT[:, :, c0:c0 + pc])

        # ---------- LN stats: bf16 casts + ones^T matmuls (replicated) ----------
        def ln_stats(src):
            xb = sqpool.tile([P, KT, PC2], BF16, tag="xb")
            xsq = sqpool.tile([P, KT, PC2], BF16, tag="xsq")
            for k in range(KT):
                nc.vector.tensor_copy(out=xb[:, k, :pc], in_=src[:, k, :pc])
                nc.scalar.activation(out=xsq[:, k, :pc], in_=src[:, k, :pc],
                                     func=AF.Square, bias=0.0, scale=1.0)
            S = ps_st.tile([P, PC2], F32, tag="S")
            Q = ps_st.tile([P, PC2], F32, tag="Q")
            for k in range(KT):
                nc.tensor.matmul(S[:, :pc], lhsT=ones_b[:], rhs=xb[:, k, :pc],
                                 start=(k == 0), stop=(k == KT - 1),
                                 skip_group_check=True)
            for k in range(KT):
                nc.tensor.matmul(Q[:, :pc], lhsT=ones_b[:], rhs=xsq[:, k, :pc],
                                 start=(k == 0), stop=(k == KT - 1),
                                 skip_group_check=True)
            return S, Q

        def ln_tail(S, Q, n, epscol, ab_scale, tag, want_cb=True):
            # rstd = n / sqrt(n*Q - S^2 + n^2 eps); Ab = ab_scale * rstd
            # tags shared across the three LNs (sequential use) to save SBUF
            tag = ""
            t0 = lnpool.tile([P, PC2], F32, tag=tag + "t0")
            nc.scalar.activation(out=t0[:, :pc], in_=S[:, :pc],
                                 func=AF.Square, bias=0.0, scale=1.0)
            nc.vector.scalar_tensor_tensor(out=t0[:, :pc], in0=Q[:, :pc], scalar=float(n),
                                           in1=t0[:, :pc], op0=OP.mult, op1=OP.subtract)
            nc.scalar.activation(out=t0[:, :pc], in_=t0[:, :pc], func=AF.Sqrt,
                                 bias=epscol[:], scale=1.0)
            if USE_RECIP_FAST:
                nc.vector.reciprocal_approx_fast(out=t0[:, :pc], in_=t0[:, :pc])
            else:
                nc.vector.reciprocal(out=t0[:, :pc], in_=t0[:, :pc])
            Ab = lnpool.tile([P, PC2], F32, tag=tag + "ab")
            nc.vector.tensor_scalar_mul(out=Ab[:, :pc], in0=t0[:, :pc],
                                        scalar1=float(n) * ab_scale)
            if not want_cb:
                return Ab, None
            Cb = lnpool.tile([P, PC2], F32, tag=tag + "cb")
            nc.vector.tensor_scalar_mul(out=Cb[:, :pc], in0=S[:, :pc],
                                        scalar1=1.0 / float(n))
            return Ab, Cb

        def normalize(src, Ab, Cb, dst, dt):
            for k in range(KT):
                tmp = nrmpool.tile([P, PC2], F32, tag="nrm")
                nc.vector.tensor_tensor(out=tmp[:, :pc], in0=src[:, k, :pc],
                                        in1=Cb[:, :pc], op=OP.subtract)
                nc.vector.tensor_tensor(out=dst[:, k, :pc], in0=tmp[:, :pc],
                                        in1=Ab[:, :pc], op=OP.mult)

        if phase == "dma":
            return
        S1, Q1 = ln_stats(x_t)
        Ab1, Cb1 = ln_tail(S1, Q1, DIM, eps1, SA, "l1")
        h1 = hpool.tile([P, KT, 512], F8E4, tag="h1")
        nc.gpsimd.memset(h1[:, :, pc:], 0.0)
        normalize(x_t, Ab1, Cb1, h1, F8E4)

        if phase == "ln":
            return
        # ---------- q/k projections (fp8 DoubleRow) + RoPE ----------
        def dr_group(psum, wmat, rhs_t, m, extra_bias=None, cols=None):
            """Accumulate wmat[:, :, m*128:(m+1)*128]^T @ rhs over K in one
            psum bank using DoubleRow; col-chunks share the group via the
            overwrite-where-unset semantics of start=False."""
            cols = chunks if cols is None else cols
            nkp = wmat.shape[1] // 2
            last = (len(cols) - 1, nkp - 1)
            for ci, (cc0, ccn) in enumerate(cols):
                for kp in range(nkp):
                    st = (kp == 0)
                    sp = (ci, kp) == last and extra_bias is None
                    nc.tensor.matmul(psum[:, cc0:cc0 + ccn],
                                     lhsT=wmat[:, 2 * kp:2 * kp + 2, m * P:(m + 1) * P],
                                     rhs=rhs_t[:, 2 * kp:2 * kp + 2, cc0:cc0 + ccn],
                                     start=st, stop=sp, perf_mode=DRM,
                                     skip_group_check=True)
            if extra_bias is not None:
                nc.tensor.matmul(psum[:, :pc], lhsT=extra_bias[:, m * P:(m + 1) * P],
                                 rhs=onesrow[:, :pc], start=False, stop=True,
                                 skip_group_check=True)

        def emit_qk(wmat, brow, dest, ds):
            for m in range(KT):
                ps = ps_mm.tile([P, PC2], F32, tag="mm")
                dr_group(ps, wmat, h1, m, extra_bias=brow)
                qs = qspool.tile([P, PC2], BF16, tag="qs")
                nc.scalar.activation(out=qs[:, :pc], in_=ps[:, :pc],
                                     func=AF.Copy, bias=0.0, scale=ds)
                rot = ps_mm.tile([P, PC2], F32, tag="mm")
                nc.tensor.matmul(rot[:, :pc], lhsT=r2t[:], rhs=qs[:, :pc],
                                 start=True, stop=True)
                t1 = qspool.tile([P, PC2], BF16, tag="t1")
                nc.vector.tensor_tensor(out=t1[:, :pc], in0=qs[:, :pc],
                                        in1=cos2[:, :pc], op=OP.mult)
                t2 = qspool.tile([P, PC2], BF16, tag="t2")
                nc.vector.tensor_tensor(out=t2[:, :pc], in0=rot[:, :pc],
                                        in1=sin2[:, :pc], op=OP.mult)
                nc.vector.tensor_tensor(out=dest[:, m, :pc], in0=t1[:, :pc],
                                        in1=t2[:, :pc], op=OP.add)

        qhat = qkpool.tile([P, KT, PC2], BF16, tag="qhat")
        khat = qkpool.tile([P, KT, PC2 + KPAD], BF16, tag="khat")
        nc.gpsimd.memset(khat[:, :, pc:], 0.0)
        emit_qk(wq, qb, qhat, sc["ds_q"])
        emit_qk(wk, kb, khat, sc["ds_k"])

        if phase == "qk":
            return
        # ---------- V (token-major, fp8 DoubleRow) ----------
        v_ts = []
        for wi in range(nwin):
            wcol = wi * NTOK
            v_t = vpool.tile([P, HEADS, 2, HD + 1], BF16, tag=f"v{wi}")
            nc.gpsimd.memset(v_t[64:128, :, 1, :], 0.0)
            for ci, (cs, cn) in enumerate([(0, 128), (128, 68)]):
                for half in range(2):
                    ps = ps_mm.tile([P, PC2], F32, tag="mm")
                    for q4 in range(2):
                        for kp in range(3):
                            st = (kp == 0)
                            sp = (q4 == 1 and kp == 2) and vb is None
                            nc.tensor.matmul(
                                ps[:, q4 * 192:(q4 + 1) * 192],
                                lhsT=h1[:, 2 * kp:2 * kp + 2, wcol + cs:wcol + cs + 128],
                                rhs=wv[:, 2 * kp:2 * kp + 2,
                                       half * 384 + q4 * 192:half * 384 + (q4 + 1) * 192],
                                start=st, stop=sp, perf_mode=DRM,
                                skip_group_check=True)
                    if vb is not None:
                        nc.tensor.matmul(ps[:, 0:384], lhsT=onesrow[:, 0:128],
                                         rhs=vb[:, half * 384:(half + 1) * 384],
                                         start=False, stop=True, skip_group_check=True)
                    nc.scalar.activation(
                        out=v_t[0:cn, half * 6:(half + 1) * 6, ci, 0:HD],
                        in_=ps[0:cn, 0:384].rearrange("p (h d) -> p h d", d=HD),
                        func=AF.Copy, bias=0.0, scale=sc["ds_v"])
            nc.gpsimd.memset(v_t[:, :, 0, HD:HD + 1], 0.0625)
            nc.vector.memset(v_t[0:68, :, 1, HD:HD + 1], 0.0625)
            v_ts.append(v_t)

        if phase == "v":
            return
        # ---------- attention ----------
        ohat = None
        if phase not in ("att1", "att2", "att3a", "att3b"):
            ohat = opool.tile([P, KT, PC2], F8E4, tag="ohat")
        for wi in range(nwin):
            wcol = wi * NTOK
            v_t = v_ts[wi]

            def head_exp(hh):
                r0 = 64 * (hh % 2)
                g6 = hh // 2
                sps = ps_sc.tile([P, PC2], F32, tag="sc")
                nc.tensor.matmul(sps[:, 0:196],
                                 lhsT=khat[r0:r0 + 64, g6, wcol:wcol + 128],
                                 rhs=qhat[r0:r0 + 64, g6, wcol:wcol + NTOK],
                                 start=True, stop=True, skip_group_check=True)
                nc.tensor.matmul(sps[:, 196:392],
                                 lhsT=khat[r0:r0 + 64, g6, wcol + 128:wcol + 256],
                                 rhs=qhat[r0:r0 + 64, g6, wcol:wcol + NTOK],
                                 start=True, stop=True, skip_group_check=True)
                e2 = epool.tile([P, 2, NTOK], BF16, tag="e2")
                nc.scalar.activation(out=e2[:].rearrange("p a b -> p (a b)"),
                                     in_=sps[:, :], func=AF.Exp, bias=eshc[:], scale=1.0)
                return e2

            for (ha, hc) in AV_PAIRS:
                ea = head_exp(ha)
                ec = head_exp(hc)
                if phase == "att1":
                    continue
                avp = ps_av.tile([P, PC2], F32, tag="av")
                for si, (hh, ee) in enumerate(((ha, ea), (hc, ec))):
                    nc.tensor.matmul(avp[0:HD + 1, si * 196:(si + 1) * 196],
                                     lhsT=v_t[:, hh, 0, :], rhs=ee[:, 0, :],
                                     start=True, stop=False, skip_group_check=True)
                    nc.tensor.matmul(avp[0:HD + 1, si * 196:(si + 1) * 196],
                                     lhsT=v_t[:, hh, 1, :], rhs=ee[:, 1, :],
                                     start=False, stop=True, skip_group_check=True)
                if phase == "att2":
                    continue
                zc = zpool.tile([1, PC2], F32, tag="zc")
                nc.vector.reciprocal(out=zc[:], in_=avp[HD:HD + 1, :])
                if phase == "att3a":
                    continue
                zb = zpool.tile([64, PC2], F32, tag="zb")
                zap = zc[:]
                nc.sync.dma_start(zb[:], bass.AP(tensor=zap.tensor, offset=zap.offset,
                                                 ap=[zap.ap[0], [0, 64], zap.ap[1]]))
                if phase == "att3b":
                    continue
                r0 = 64 * (ha % 2)
                g6 = ha // 2
                nc.vector.tensor_tensor(
                    out=ohat[r0:r0 + 64, g6:g6 + 2, wcol:wcol + NTOK],
                    in0=avp[0:64, :].rearrange("p (a b) -> p a b", a=2),
                    in1=zb[:].rearrange("p (a b) -> p a b", a=2),
                    op=OP.mult)

        if phase in ("att", "att1", "att2", "att3a", "att3b"):
            return
        # ---------- proj + residual ----------
        x1 = x1pool.tile([P, KT, PC2], F32, tag="x1")
        for m in range(KT):
            ps = ps_mm.tile([P, PC2], F32, tag="mm")
            dr_group(ps, wp, ohat, m, extra_bias=pb)
            nc.vector.scalar_tensor_tensor(out=x1[:, m, :pc], in0=ps[:, :pc],
                                           scalar=sc["ds_p"], in1=x_t[:, m, :pc],
                                           op0=OP.mult, op1=OP.add)

        if phase == "proj":
            return
        # ---------- LN2 + MLP ----------
        S2, Q2 = ln_stats(x1)
        Ab2, Cb2 = ln_tail(S2, Q2, DIM, eps1, 1.0, "l2")
        h2 = hpool.tile([P, KT, PC2], BF16, tag="h2")
        normalize(x1, Ab2, Cb2, h2, BF16)

        g = gpool.tile([P, MT, PC2], BF16, tag="g")
        sg = ps_st.tile([P, PC2], F32, tag="S")
        qg = ps_st.tile([P, PC2], F32, tag="Q")
        for m in range(MT):
            p1 = ps_mm.tile([P, PC2], F32, tag="mm")
            for k in range(KT):
                nc.tensor.matmul(p1[:, :pc], lhsT=w1[:, k, m * P:(m + 1) * P],
                                 rhs=h2[:, k, :pc], start=(k == 0),
                                 stop=(k == KT - 1 and w1b is None))
            if w1b is not None:
                nc.tensor.matmul(p1[:, :pc], lhsT=w1b[:, m * P:(m + 1) * P],
                                 rhs=onesrow[:, :pc], start=False, stop=True,
                                 skip_group_check=True)
            sf = mlppool.tile([P, PC2], BF16, tag="sf")
            if USE_SILU:
                nc.scalar.activation(out=sf[:, :pc], in_=p1[:, :pc], func=AF.Silu,
                                     bias=0.0, scale=1.0)
            else:
                s1 = mlppool.tile([P, PC2], BF16, tag="s1")
                nc.scalar.activation(out=s1[:, :pc], in_=p1[:, :pc], func=AF.Sigmoid,
                                     bias=0.0, scale=1.0)
                nc.vector.tensor_tensor(out=sf[:, :pc], in0=p1[:, :pc],
                                        in1=s1[:, :pc], op=OP.mult)
            p2 = ps_mm.tile([P, PC2], F32, tag="mm")
            for k in range(KT):
                nc.tensor.matmul(p2[:, :pc], lhsT=w2[:, k, m * P:(m + 1) * P],
                                 rhs=h2[:, k, :pc], start=(k == 0),
                                 stop=(k == KT - 1 and w2b is None))
            if w2b is not None:
                nc.tensor.matmul(p2[:, :pc], lhsT=w2b[:, m * P:(m + 1) * P],
                                 rhs=onesrow[:, :pc], start=False, stop=True,
                                 skip_group_check=True)
            nc.vector.tensor_tensor(out=g[:, m, :pc], in0=p2[:, :pc],
                                    in1=sf[:, :pc], op=OP.mult)
            sqg = mlppool.tile([P, PC2], BF16, tag="sqg")
            nc.vector.tensor_tensor(out=sqg[:, :pc], in0=g[:, m, :pc],
                                    in1=g[:, m, :pc], op=OP.mult)
            nc.tensor.matmul(sg[:, :pc], lhsT=ones_b[:], rhs=g[:, m, :pc],
                             start=(m == 0), stop=(m == MT - 1), skip_group_check=True)
            nc.tensor.matmul(qg[:, :pc], lhsT=ones_b[:], rhs=sqg[:, :pc],
                             start=(m == 0), stop=(m == MT - 1), skip_group_check=True)

        if phase == "mlp":
            return
        At, _ = ln_tail(sg, qg, HID, eps3, 1.0, "l3", want_cb=False)
        mrow = lnpool.tile([1, PC2], BF16, tag="mrow")
        nc.vector.tensor_scalar_mul(out=mrow[:, :pc], in0=sg[0:1, :pc],
                                    scalar1=1.0 / float(HID))

        # ---------- w3 + hid-LN scale + residual ----------
        for m in range(KT):
            ps = ps_mm.tile([P, PC2], F32, tag="mm")
            for k in range(MT):
                nc.tensor.matmul(ps[:, :pc], lhsT=w3[:, k, m * P:(m + 1) * P],
                                 rhs=g[:, k, :pc], start=(k == 0), stop=False,
                                 skip_group_check=True)
            nc.tensor.matmul(ps[:, :pc], lhsT=w3csr[:, m * P:(m + 1) * P],
                             rhs=mrow[:, :pc], start=False, stop=True,
                             skip_group_check=True)
            yt = ypool.tile([P, PC2], F32, tag="yt")
            nc.vector.tensor_tensor(out=yt[:, :pc], in0=ps[:, :pc],
                                    in1=At[:, :pc], op=OP.mult)
            if w3bc is not None:
                nc.vector.scalar_tensor_tensor(out=yt[:, :pc], in0=yt[:, :pc],
                                               scalar=w3bc[:, m:m + 1],
                                               in1=x1[:, m, :pc], op0=OP.add, op1=OP.add)
            else:
                nc.vector.tensor_tensor(out=yt[:, :pc], in0=yt[:, :pc],
                                        in1=x1[:, m, :pc], op=OP.add)
            nc.sync.dma_start(yT[:, m, c0:c0 + pc], yt[:, :pc])

    def emit_all():
        for wins in pairs:
            emit_pair(wins)
        if phase != "full":
            yt0 = ypool.tile([P, PC2], F32, tag="yt")
            nc.vector.memset(yt0[:], 0.0)
            nc.sync.dma_start(yT[:, 0, 0:PC2], yt0[:])

    if loop_n > 1:
        with tc.For_i(0, loop_n, 1):
            emit_all()
    else:
        emit_all()


def _build(has_biases, nwin_total=NWIN, ncores=N_CORES, loop_n=1, scales=None, phase='full'):
    key = ("prog", USE_SILU, USE_RECIP_FAST, tuple(sorted(has_biases.items())), nwin_total, ncores, loop_n,
           tuple(sorted((scales or {}).items())), phase)
    if key in _cache:
        return _cache[key]
    nc = bacc.Bacc("TRN2", target_bir_lowering=False, debug=False,
                   enable_asserts=False, num_devices=ncores)
    toks = nwin_total * NTOK
    aps = {}
    aps["xT"] = nc.dram_tensor("xT", [DIM, toks], F32, kind="ExternalInput").ap()
    aps["yT"] = nc.dram_tensor("yT", [DIM, toks], F32, kind="ExternalOutput").ap()
    for nm, shp, dt in [("wq", [DIM, DIM], F8E4), ("wk", [DIM, DIM], F8E4),
                        ("wv", [DIM, DIM], F8E4), ("wp", [DIM, DIM], F8E4),
                        ("w1", [DIM, HID], BF16), ("w2", [DIM, HID], BF16),
                        ("w3", [HID, DIM], BF16)]:
        aps[nm] = nc.dram_tensor(nm, shp, dt, kind="ExternalInput").ap()
    aps["cos2"] = nc.dram_tensor("cos2", [P, PC2], BF16, kind="ExternalInput").ap()
    aps["sin2"] = nc.dram_tensor("sin2", [P, PC2], BF16, kind="ExternalInput").ap()
    aps["r2t"] = nc.dram_tensor("r2t", [P, P], BF16, kind="ExternalInput").ap()
    aps["w3csr"] = nc.dram_tensor("w3csr", [1, DIM], BF16, kind="ExternalInput").ap()
    for nm, d in [("qb", DIM), ("kb", DIM), ("vb", DIM), ("pb", DIM),
                  ("w1b", HID), ("w2b", HID)]:
        aps[nm] = (nc.dram_tensor(nm, [1, d], BF16, kind="ExternalInput").ap()
                   if has_biases.get(nm) else None)
    aps["w3b"] = (nc.dram_tensor("w3b", [DIM], F32, kind="ExternalInput").ap()
                  if has_biases.get("w3b") else None)
    sc = scales or {"ds_q": 1.0, "ds_k": 1.0, "ds_v": 1.0, "ds_p": 1.0}
    with tile.TileContext(nc) as tc:
        with ExitStack() as ctx:
            _emit(nc, tc, ctx, aps, has_biases, sc, nwin_total, loop_n, phase)
    nc.compile()
    _cache[key] = nc
    return nc


def _pick_scale(w, target=160.0):
    a = float(np.abs(w).max())
    return 2.0 ** np.floor(np.log2(target / a)) if a > 0 else 1.0


def _host_prep(inputs):
    f = {k: np.asarray(v, np.float32) if hasattr(v, "shape") else v
         for k, v in inputs.items()}
    scale = HD ** -0.5
    wq = f["ln1_w"][:, None] * f["q_w"] * scale
    wk = f["ln1_w"][:, None] * f["k_w"]
    wv = f["ln1_w"][:, None] * f["v_w"]
    qb = (f["ln1_b"] @ f["q_w"] + f["q_b"]) * scale
    kb = f["ln1_b"] @ f["k_w"]
    vb = f["ln1_b"] @ f["v_w"] + f["v_b"]
    wp = f["proj_w"]
    pb = f["proj_b"]
    w1 = f["ln2_w"][:, None] * f["w1_w"]
    w2 = f["ln2_w"][:, None] * f["w2_w"]
    w1b = f["ln2_b"] @ f["w1_w"] + f["w1_b"]
    w2b = f["ln2_b"] @ f["w2_w"] + f["w2_b"]
    w3 = f["ffn_w"][:, None] * f["w3_w"]
    w3b = f["ffn_b"] @ f["w3_w"] + f["w3_b"]

    sq, sk, sv, sp = (_pick_scale(wq), _pick_scale(wk),
                      _pick_scale(wv), _pick_scale(wp))
    scales = {"ds_q": 1.0 / (sq * SA), "ds_k": 1.0 / (sk * SA),
              "ds_v": 1.0 / (sv * SA), "ds_p": 1.0 / (sp * SV)}

    cos, sin = _rope_tables()
    cosT = np.ascontiguousarray(cos.T)
    sinT = np.ascontiguousarray(sin.T)
    cos2 = np.tile(np.concatenate([cosT, cosT], 0), (1, 2))
    sin2 = np.tile(np.concatenate([sinT, sinT], 0), (1, 2))

    r = np.zeros((64, 64), np.float32)
    for i in range(32):
        r[2 * i, 2 * i + 1] = -1.0
        r[2 * i + 1, 2 * i] = 1.0
    r2 = np.zeros((128, 128), np.float32)
    r2[:64, :64] = r
    r2[64:, 64:] = r
    r2t = np.ascontiguousarray(r2.T)

    x = f["x"]
    pad = (-H) % WS
    nw = (H + pad) // WS
    xp = np.pad(x, ((0, 0), (0, pad), (0, pad), (0, 0)))
    t = xp.reshape(B, nw, WS, nw, WS, DIM).transpose(0, 1, 3, 2, 4, 5).reshape(B, NWIN * NTOK, DIM)

    shared = {
        "wq": (wq * sq).astype(F8E4NP), "wk": (wk * sk).astype(F8E4NP),
        "wv": (wv * sv).astype(F8E4NP), "wp": (wp * sp).astype(F8E4NP),
        "w1": w1.astype(BF16NP), "w2": w2.astype(BF16NP), "w3": w3.astype(BF16NP),
        "cos2": cos2.astype(BF16NP), "sin2": sin2.astype(BF16NP),
        "r2t": r2t.astype(BF16NP),
        "w3csr": np.ascontiguousarray(-w3.sum(0)[None, :]).astype(BF16NP),
    }
    # biases pre-scaled into psum units of their group
    brows = {"qb": qb * (sq * SA), "kb": kb * (sk * SA), "vb": vb * (sv * SA),
             "pb": pb * (sp * SV), "w1b": w1b, "w2b": w2b}
    has_biases = {k: bool(np.any(np.asarray(v) != 0.0)) for k, v in brows.items()}
    has_biases["w3b"] = bool(np.any(w3b != 0.0))
    for k, v in brows.items():
        if has_biases[k]:
            shared[k] = np.ascontiguousarray(v[None, :]).astype(BF16NP)
    if has_biases["w3b"]:
        shared["w3b"] = np.ascontiguousarray(w3b, np.float32)

    in_maps = []
    for b in range(B):
        m = dict(shared)
        m["xT"] = np.ascontiguousarray(t[b].T)
        in_maps.append(m)
    return in_maps, has_biases, scales


def _host_post(results):
    pad = (-H) % WS
    nw = (H + pad) // WS
    Hp = H + pad
    y = np.empty((B, H, W, DIM), np.float32)
    for b in range(B):
        yb = np.asarray(results[b]["yT"])
        yw = yb.T.reshape(nw, nw, WS, WS, DIM).transpose(0, 2, 1, 3, 4).reshape(Hp, Hp, DIM)
        y[b] = yw[:H, :W, :]
    return y


def kernel(**inputs):
    in_maps, has_biases, scales = _host_prep(inputs)
    nc = _build(has_biases, scales=scales)
    res = run_bass_kernel_spmd(nc, in_maps, core_ids=list(range(N_CORES)))
    return _host_post(res.results)


# revision 32
# speedup vs baseline: 1.5731x; 1.3418x over previous
"""Trainium2 Bass kernel for a Swin-style transformer block
(windowed attention with RoPE + SwiGLU MLP with sub-LN).

Sharding: data-parallel over batch B=8 -> one image per NeuronCore.

v2 scheme (vs v1 baseline):
- fp8(e4m3) + DoubleRow matmuls for q/k/v/proj GEMMs (2x PE throughput);
  attention probs in fp8(e5m2) with a constant -5 exp shift (cancels in
  softmax), AV matmul in DoubleRow over the two 128/68 key chunks with
  zero-padded V rows annihilating pad/fake keys.
- MLP (w1/w2/w3) stays bf16: fp8 there fails the 2e-2 gate (measured
  3.3e-2 host-side).
- LN1/LN2 stats via fp32r ones-matmuls on x / x^2 (replicated psum out,
  no gpsimd all-reduce); rstd via sqrt(Act) + fast-reciprocal(DVE).
- hid-LN mean folded into w3 via an extra (-colsum(w3), mean) rank-1
  matmul row; A applied on the w3 output.
- Scores for both key chunks land in one PSUM bank -> single Exp per
  window-head; single DoubleRow AV per window-head; per-head-pair z
  reciprocal + broadcast-DMA + one normalize mult.
- Elementwise work split across DVE / Act / Pool (engine map below).

Numerics vs fp32 reference (host emulation): ~3.5e-3 absmax-rel.
"""
import numpy as np
import ml_dtypes
from contextlib import ExitStack

import concourse.bass as bass
import concourse.tile as tile
from concourse import bacc, mybir
from concourse.bass_utils import run_bass_kernel_spmd

BF16NP = ml_dtypes.bfloat16
F8E4NP = ml_dtypes.float8_e4m3
F8E5NP = ml_dtypes.float8_e5m2
F32 = mybir.dt.float32
F32R = mybir.dt.float32r
BF16 = mybir.dt.bfloat16
F8E4 = mybir.dt.float8e4
F8E5 = mybir.dt.float8e5
OP = mybir.AluOpType
AF = mybir.ActivationFunctionType
DRM = mybir.MatmulPerfMode.DoubleRow

DIM = 768
HEADS = 12
HD = 64
HID = 2048
EPS = 1e-6
WS = 14
NTOK = WS * WS          # 196
B, H, W = 8, 64, 64
NWIN = 25
TOKS = NWIN * NTOK      # 4900
KT = DIM // 128         # 6
MT = HID // 128         # 16
N_CORES = 8
P = 128
PC2 = 2 * NTOK          # 392
KPAD = 64               # khat column pad so chunk-1 lhsT always in-bounds
SA = 16.0               # h1 fp8 scale
SV = 16.0               # v / ohat fp8 scale
ESH = -5.0              # exp shift (cancels in softmax)

USE_SILU = True          # CoreSim lacks Silu; simtest flips this off
USE_RECIP_FAST = True    # fall back to nc.vector.reciprocal if False

_cache = {}


def _rope_tables():
    dim, pt, theta = 32, 16.0, 10000.0
    freqs = 1.0 / theta ** (np.arange(0, dim, 2, dtype=np.float32) / dim)
    f1 = np.repeat((np.arange(WS, dtype=np.float32) / WS * pt)[:, None] * freqs[None, :], 2, axis=-1)
    f = np.concatenate([
        np.broadcast_to(f1[:, None, :], (WS, WS, dim)),
        np.broadcast_to(f1[None, :, :], (WS, WS, dim)),
    ], -1).reshape(NTOK, 2 * dim)
    return np.cos(f), np.sin(f)   # [196, 64]


# head-pair banks for AV: (h, h+2) share a psum bank, adjacent k-tiles
AV_PAIRS = [(0, 2), (1, 3), (4, 6), (5, 7), (8, 10), (9, 11)]


def _emit(nc, tc, ctx, aps, hb, sc, nwin_total=NWIN, loop_n=1, phase='full'):
    pairs = []
    w = 0
    while w < nwin_total:
        pairs.append((w, w + 1) if w + 1 < nwin_total else (w,))
        w += 2

    xT = aps["xT"].rearrange("(k p) n -> p k n", p=P)
    yT = aps["yT"].rearrange("(k p) n -> p k n", p=P)

    consts = ctx.enter_context(tc.tile_pool(name="consts", bufs=1))
    wpool = ctx.enter_context(tc.tile_pool(name="weights", bufs=1))
    xpool = ctx.enter_context(tc.tile_pool(name="x", bufs=2))
    sqpool = ctx.enter_context(tc.tile_pool(name="xsq", bufs=3))
    lnpool = ctx.enter_context(tc.tile_pool(name="ln", bufs=1))
    nrmpool = ctx.enter_context(tc.tile_pool(name="nrm", bufs=2))
    hpool = ctx.enter_context(tc.tile_pool(name="h", bufs=1))
    qspool = ctx.enter_context(tc.tile_pool(name="qs", bufs=2))
    qkpool = ctx.enter_context(tc.tile_pool(name="qk", bufs=1))
    kkpool = ctx.enter_context(tc.tile_pool(name="kk", bufs=2))
    vpool = ctx.enter_context(tc.tile_pool(name="v", bufs=1))
    epool = ctx.enter_context(tc.tile_pool(name="e", bufs=3))
    zpool = ctx.enter_context(tc.tile_pool(name="z", bufs=3))
    opool = ctx.enter_context(tc.tile_pool(name="ohat", bufs=1))
    x1pool = ctx.enter_context(tc.tile_pool(name="x1", bufs=1))
    mlppool = ctx.enter_context(tc.tile_pool(name="mlp", bufs=2))
    gpool = ctx.enter_context(tc.tile_pool(name="g", bufs=1))
    ypool = ctx.enter_context(tc.tile_pool(name="y", bufs=2))

    ps_mm = ctx.enter_context(tc.tile_pool(name="psmm", bufs=2, space="PSUM"))
    ps_sc = ctx.enter_context(tc.tile_pool(name="pssc", bufs=2, space="PSUM"))
    ps_av = ctx.enter_context(tc.tile_pool(name="psav", bufs=2, space="PSUM"))
    ps_st = ctx.enter_context(tc.tile_pool(name="psst", bufs=1, space="PSUM"))

    # ---- weights ----
    def load_w(name, kdim, mdim, dt):
        t = wpool.tile([P, kdim // P, mdim], dt, tag=name)
        nc.sync.dma_start(t[:], aps[name].rearrange("(k p) m -> p k m", p=P))
        return t

    wq = load_w("wq", DIM, DIM, F8E4)
    wk = load_w("wk", DIM, DIM, F8E4)
    wv = load_w("wv", DIM, DIM, F8E4)
    wp = load_w("wp", DIM, DIM, F8E4)
    w1 = load_w("w1", DIM, HID, BF16)
    w2 = load_w("w2", DIM, HID, BF16)
    w3 = load_w("w3", HID, DIM, BF16)

    cos2 = consts.tile([P, PC2], BF16, tag="cos2")
    nc.sync.dma_start(cos2[:], aps["cos2"][:])
    sin2 = consts.tile([P, PC2], BF16, tag="sin2")
    nc.sync.dma_start(sin2[:], aps["sin2"][:])
    r2t = consts.tile([P, P], BF16, tag="r2t")
    nc.sync.dma_start(r2t[:], aps["r2t"][:])
    w3csr = consts.tile([1, DIM], BF16, tag="w3csr")   # -colsum(w3)
    nc.sync.dma_start(w3csr[:], aps["w3csr"][:])

    ones_b = consts.tile([P, P], BF16, tag="ones_b")
    nc.vector.memset(ones_b[:], 1.0)
    onesrow = consts.tile([1, PC2], BF16, tag="onesrow")
    nc.vector.memset(onesrow[:], 1.0)
    eps1 = consts.tile([P, 1], F32, tag="eps1")
    nc.vector.memset(eps1[:], float(DIM) * float(DIM) * EPS)
    eps3 = consts.tile([P, 1], F32, tag="eps3")
    nc.vector.memset(eps3[:], float(HID) * float(HID) * EPS)
    eshc = consts.tile([P, 1], F32, tag="eshc")
    nc.vector.memset(eshc[:], ESH)

    def bias_row(name, feat):
        if aps.get(name) is None:
            return None
        t = consts.tile([1, feat], BF16, tag=name)
        nc.sync.dma_start(t[:], aps[name][:])
        return t

    qb = bias_row("qb", DIM)      # pre-scaled on host to psum units
    kb = bias_row("kb", DIM)
    vb = bias_row("vb", DIM)
    pb = bias_row("pb", DIM)
    w1b = bias_row("w1b", HID)
    w2b = bias_row("w2b", HID)
    w3b = aps.get("w3b") is not None
    w3bc = None
    if w3b:
        w3bc = consts.tile([P, KT], F32, tag="w3bc")
        nc.sync.dma_start(w3bc[:], aps["w3b"].rearrange("(k p) -> p k", p=P))

    def emit_pair(wins):
        nwin = len(wins)
        pc = NTOK * nwin
        c0 = wins[0] * NTOK
        chunks = [(0, 196)] if nwin == 1 else [(0, 196), (196, 196)]

        x_t = xpool.tile([P, KT, PC2], F32, tag="x")
        nc.sync.dma_start(x_t[:, :, :pc], xT[:, :, c0:c0 + pc])

        _mmi = [0]

        def mmtile():
            _mmi[0] += 1
            r = _mmi[0] % 3
            if r == 0:
                return ps_mm.tile([P, PC2], F32, tag="mm", name="mmt")
            if r == 1:
                return ps_sc.tile([P, PC2], F32, tag="sc", name="sct")
            return ps_av.tile([P, PC2], F32, tag="av", name="avt")

        # ---------- LN stats: bf16 casts + ones^T matmuls (replicated) ----------
        def ln_stats(src):
            S = ps_st.tile([P, PC2], F32, tag="S")
            Q = ps_st.tile([P, PC2], F32, tag="Q")
            for k in range(KT):
                xb = sqpool.tile([P, PC2], BF16, tag="xb")
                nc.vector.tensor_copy(out=xb[:, :pc], in_=src[:, k, :pc])
                xsq = sqpool.tile([P, PC2], BF16, tag="xsq")
                nc.vector.tensor_tensor(out=xsq[:, :pc], in0=src[:, k, :pc],
                                        in1=src[:, k, :pc], op=OP.mult)
                nc.tensor.matmul(S[:, :pc], lhsT=ones_b[:], rhs=xb[:, :pc],
                                 start=(k == 0), stop=(k == KT - 1),
                                 skip_group_check=True)
                nc.tensor.matmul(Q[:, :pc], lhsT=ones_b[:], rhs=xsq[:, :pc],
                                 start=(k == 0), stop=(k == KT - 1),
                                 skip_group_check=True)
            return S, Q

        def ln_tail(S, Q, n, epscol, ab_scale, tag, want_cb=True):
            # rstd = n / sqrt(n*Q - S^2 + n^2 eps); Ab = ab_scale * rstd
            # tags shared across the three LNs (sequential use) to save SBUF
            tag = ""
            t0 = lnpool.tile([P, PC2], F32, tag=tag + "t0")
            nc.scalar.activation(out=t0[:, :pc], in_=S[:, :pc],
                                 func=AF.Square, bias=0.0, scale=1.0)
            nc.vector.scalar_tensor_tensor(out=t0[:, :pc], in0=Q[:, :pc], scalar=float(n),
                                           in1=t0[:, :pc], op0=OP.mult, op1=OP.subtract)
            nc.scalar.activation(out=t0[:, :pc], in_=t0[:, :pc], func=AF.Sqrt,
                                 bias=epscol[:], scale=1.0)
            if USE_RECIP_FAST:
                nc.vector.reciprocal_approx_fast(out=t0[:, :pc], in_=t0[:, :pc])
            else:
                nc.vector.reciprocal(out=t0[:, :pc], in_=t0[:, :pc])
            Ab = lnpool.tile([P, PC2], F32, tag=tag + "ab")
            nc.vector.tensor_scalar_mul(out=Ab[:, :pc], in0=t0[:, :pc],
                                        scalar1=float(n) * ab_scale)
            if not want_cb:
                return Ab, None
            Cb = lnpool.tile([P, PC2], F32, tag=tag + "cb")
            nc.vector.tensor_scalar_mul(out=Cb[:, :pc], in0=S[:, :pc],
                                        scalar1=1.0 / float(n))
            return Ab, Cb

        def normalize(src, Ab, Cb, dst, dt):
            for k in range(KT):
                tmp = nrmpool.tile([P, PC2], F32, tag="nrm")
                nc.vector.tensor_tensor(out=tmp[:, :pc], in0=src[:, k, :pc],
                                        in1=Cb[:, :pc], op=OP.subtract)
                nc.vector.tensor_tensor(out=dst[:, k, :pc], in0=tmp[:, :pc],
                                        in1=Ab[:, :pc], op=OP.mult)

        if phase == "dma":
            return
        S1, Q1 = ln_stats(x_t)
        Ab1, Cb1 = ln_tail(S1, Q1, DIM, eps1, SA, "l1")
        h1 = hpool.tile([P, KT, 512], F8E4, tag="h1")
        nc.gpsimd.memset(h1[:, :, pc:], 0.0)
        normalize(x_t, Ab1, Cb1, h1, F8E4)

        if phase == "ln":
            return
        # ---------- q/k projections (fp8 DoubleRow) + RoPE ----------
        def dr_group(psum, wmat, rhs_t, m, extra_bias=None, cols=None):
            """Accumulate wmat[:, :, m*128:(m+1)*128]^T @ rhs over K in one
            psum bank using DoubleRow; col-chunks share the group via the
            overwrite-where-unset semantics of start=False."""
            cols = chunks if cols is None else cols
            nkp = wmat.shape[1] // 2
            last = (len(cols) - 1, nkp - 1)
            for ci, (cc0, ccn) in enumerate(cols):
                for kp in range(nkp):
                    st = (kp == 0)
                    sp = (ci, kp) == last and extra_bias is None
                    nc.tensor.matmul(psum[:, cc0:cc0 + ccn],
                                     lhsT=wmat[:, 2 * kp:2 * kp + 2, m * P:(m + 1) * P],
                                     rhs=rhs_t[:, 2 * kp:2 * kp + 2, cc0:cc0 + ccn],
                                     start=st, stop=sp, perf_mode=DRM,
                                     skip_group_check=True)
            if extra_bias is not None:
                nc.tensor.matmul(psum[:, :pc], lhsT=extra_bias[:, m * P:(m + 1) * P],
                                 rhs=onesrow[:, :pc], start=False, stop=True,
                                 skip_group_check=True)

        def emit_qk(wmat, brow, dest, ds):
            for m in range(KT):
                ps = mmtile()
                dr_group(ps, wmat, h1, m, extra_bias=brow)
                qs = qspool.tile([P, PC2], BF16, tag="qs")
                nc.scalar.activation(out=qs[:, :pc], in_=ps[:, :pc],
                                     func=AF.Copy, bias=0.0, scale=ds)
                rot = mmtile()
                nc.tensor.matmul(rot[:, :pc], lhsT=r2t[:], rhs=qs[:, :pc],
                                 start=True, stop=True)
                t1 = qspool.tile([P, PC2], BF16, tag="t1")
                nc.vector.tensor_tensor(out=t1[:, :pc], in0=qs[:, :pc],
                                        in1=cos2[:, :pc], op=OP.mult)
                t2 = qspool.tile([P, PC2], BF16, tag="t2")
                nc.vector.tensor_tensor(out=t2[:, :pc], in0=rot[:, :pc],
                                        in1=sin2[:, :pc], op=OP.mult)
                nc.vector.tensor_tensor(out=dest[:, m, :pc], in0=t1[:, :pc],
                                        in1=t2[:, :pc], op=OP.add)

        qhat = qkpool.tile([P, KT, PC2], BF16, tag="qhat")
        khat = kkpool.tile([P, KT, PC2 + KPAD], BF16, tag="khat")
        nc.gpsimd.memset(khat[:, :, pc:], 0.0)
        emit_qk(wq, qb, qhat, sc["ds_q"])
        emit_qk(wk, kb, khat, sc["ds_k"])

        if phase == "qk":
            return
        # ---------- V (token-major, fp8 DoubleRow) ----------
        v_ts = []
        for wi in range(nwin):
            wcol = wi * NTOK
            v_t = vpool.tile([P, HEADS, 2, HD + 1], BF16, tag=f"v{wi}")
            nc.gpsimd.memset(v_t[64:128, :, 1, :], 0.0)
            for ci, (cs, cn) in enumerate([(0, 128), (128, 68)]):
                for half in range(2):
                    ps = mmtile()
                    for q4 in range(2):
                        for kp in range(3):
                            st = (kp == 0)
                            sp = (q4 == 1 and kp == 2) and vb is None
                            nc.tensor.matmul(
                                ps[:, q4 * 192:(q4 + 1) * 192],
                                lhsT=h1[:, 2 * kp:2 * kp + 2, wcol + cs:wcol + cs + 128],
                                rhs=wv[:, 2 * kp:2 * kp + 2,
                                       half * 384 + q4 * 192:half * 384 + (q4 + 1) * 192],
                                start=st, stop=sp, perf_mode=DRM,
                                skip_group_check=True)
                    if vb is not None:
                        nc.tensor.matmul(ps[:, 0:384], lhsT=onesrow[:, 0:128],
                                         rhs=vb[:, half * 384:(half + 1) * 384],
                                         start=False, stop=True, skip_group_check=True)
                    nc.scalar.activation(
                        out=v_t[0:cn, half * 6:(half + 1) * 6, ci, 0:HD],
                        in_=ps[0:cn, 0:384].rearrange("p (h d) -> p h d", d=HD),
                        func=AF.Copy, bias=0.0, scale=sc["ds_v"])
            nc.gpsimd.memset(v_t[:, :, 0, HD:HD + 1], 0.0625)
            nc.vector.memset(v_t[0:68, :, 1, HD:HD + 1], 0.0625)
            v_ts.append(v_t)

        if phase == "v":
            return
        # ---------- attention ----------
        ohat = None
        if phase not in ("att1", "att2", "att3a", "att3b"):
            ohat = opool.tile([P, KT, PC2], F8E4, tag="ohat")
        for wi in range(nwin):
            wcol = wi * NTOK
            v_t = v_ts[wi]

            def head_exp(hh):
                r0 = 64 * (hh % 2)
                g6 = hh // 2
                sps = ps_sc.tile([P, PC2], F32, tag="sc")
                nc.tensor.matmul(sps[:, 0:196],
                                 lhsT=khat[r0:r0 + 64, g6, wcol:wcol + 128],
                                 rhs=qhat[r0:r0 + 64, g6, wcol:wcol + NTOK],
                                 start=True, stop=True, skip_group_check=True)
                nc.tensor.matmul(sps[:, 196:392],
                                 lhsT=khat[r0:r0 + 64, g6, wcol + 128:wcol + 256],
                                 rhs=qhat[r0:r0 + 64, g6, wcol:wcol + NTOK],
                                 start=True, stop=True, skip_group_check=True)
                e2 = epool.tile([P, 2, NTOK], BF16, tag="e2")
                nc.scalar.activation(out=e2[:].rearrange("p a b -> p (a b)"),
                                     in_=sps[:, :], func=AF.Exp, bias=eshc[:], scale=1.0)
                return e2

            for pi, (ha, hc) in enumerate(AV_PAIRS):
                ea = head_exp(ha)
                ec = head_exp(hc)
                if phase == "att1":
                    continue
                if pi == 2:
                    avp = ps_st.tile([P, PC2], F32, tag="S")
                elif pi == 3:
                    avp = ps_st.tile([P, PC2], F32, tag="Q")
                else:
                    avp = ps_av.tile([P, PC2], F32, tag="av")
                for si, (hh, ee) in enumerate(((ha, ea), (hc, ec))):
                    nc.tensor.matmul(avp[0:HD + 1, si * 196:(si + 1) * 196],
                                     lhsT=v_t[:, hh, 0, :], rhs=ee[:, 0, :],
                                     start=True, stop=False, skip_group_check=True)
                    nc.tensor.matmul(avp[0:HD + 1, si * 196:(si + 1) * 196],
                                     lhsT=v_t[:, hh, 1, :], rhs=ee[:, 1, :],
                                     start=False, stop=True, skip_group_check=True)
                if phase == "att2":
                    continue
                zc = zpool.tile([1, PC2], F32, tag="zc")
                nc.vector.reciprocal(out=zc[:], in_=avp[HD:HD + 1, :])
                if phase == "att3a":
                    continue
                zb = zpool.tile([64, PC2], F32, tag="zb")
                zap = zc[:]
                nc.sync.dma_start(zb[:], bass.AP(tensor=zap.tensor, offset=zap.offset,
                                                 ap=[zap.ap[0], [0, 64], zap.ap[1]]))
                if phase == "att3b":
                    continue
                r0 = 64 * (ha % 2)
                g6 = ha // 2
                nc.vector.tensor_tensor(
                    out=ohat[r0:r0 + 64, g6:g6 + 2, wcol:wcol + NTOK],
                    in0=avp[0:64, :].rearrange("p (a b) -> p a b", a=2),
                    in1=zb[:].rearrange("p (a b) -> p a b", a=2),
                    op=OP.mult)

        if phase in ("att", "att1", "att2", "att3a", "att3b"):
            return
        # ---------- proj + residual ----------
        x1 = x1pool.tile([P, KT, PC2], F32, tag="x1")
        for m in range(KT):
            ps = mmtile()
            dr_group(ps, wp, ohat, m, extra_bias=pb)
            nc.vector.scalar_tensor_tensor(out=x1[:, m, :pc], in0=ps[:, :pc],
                                           scalar=sc["ds_p"], in1=x_t[:, m, :pc],
                                           op0=OP.mult, op1=OP.add)

        if phase == "proj":
            return
        # ---------- LN2 + MLP ----------
        S2, Q2 = ln_stats(x1)
        Ab2, Cb2 = ln_tail(S2, Q2, DIM, eps1, 1.0, "l2")
        h2 = hpool.tile([P, KT, PC2], BF16, tag="h2")
        normalize(x1, Ab2, Cb2, h2, BF16)

        g = gpool.tile([P, MT, PC2], BF16, tag="g")
        sg = ps_st.tile([P, PC2], F32, tag="S")
        qg = ps_st.tile([P, PC2], F32, tag="Q")
        for m in range(MT):
            p1 = mmtile()
            for k in range(KT):
                nc.tensor.matmul(p1[:, :pc], lhsT=w1[:, k, m * P:(m + 1) * P],
                                 rhs=h2[:, k, :pc], start=(k == 0),
                                 stop=(k == KT - 1 and w1b is None))
            if w1b is not None:
                nc.tensor.matmul(p1[:, :pc], lhsT=w1b[:, m * P:(m + 1) * P],
                                 rhs=onesrow[:, :pc], start=False, stop=True,
                                 skip_group_check=True)
            sf = mlppool.tile([P, PC2], BF16, tag="sf")
            if USE_SILU:
                nc.scalar.activation(out=sf[:, :pc], in_=p1[:, :pc], func=AF.Silu,
                                     bias=0.0, scale=1.0)
            else:
                s1 = mlppool.tile([P, PC2], BF16, tag="s1")
                nc.scalar.activation(out=s1[:, :pc], in_=p1[:, :pc], func=AF.Sigmoid,
                                     bias=0.0, scale=1.0)
                nc.vector.tensor_tensor(out=sf[:, :pc], in0=p1[:, :pc],
                                        in1=s1[:, :pc], op=OP.mult)
            p2 = mmtile()
            for k in range(KT):
                nc.tensor.matmul(p2[:, :pc], lhsT=w2[:, k, m * P:(m + 1) * P],
                                 rhs=h2[:, k, :pc], start=(k == 0),
                                 stop=(k == KT - 1 and w2b is None))
            if w2b is not None:
                nc.tensor.matmul(p2[:, :pc], lhsT=w2b[:, m * P:(m + 1) * P],
                                 rhs=onesrow[:, :pc], start=False, stop=True,
                                 skip_group_check=True)
            nc.vector.tensor_tensor(out=g[:, m, :pc], in0=p2[:, :pc],
                                    in1=sf[:, :pc], op=OP.mult)
            sqg = mlppool.tile([P, PC2], BF16, tag="sqg")
            nc.vector.tensor_tensor(out=sqg[:, :pc], in0=g[:, m, :pc],
                                    in1=g[:, m, :pc], op=OP.mult)
            nc.tensor.matmul(sg[:, :pc], lhsT=ones_b[:], rhs=g[:, m, :pc],
                             start=(m == 0), stop=(m == MT - 1), skip_group_check=True)
            nc.tensor.matmul(qg[:, :pc], lhsT=ones_b[:], rhs=sqg[:, :pc],
                             start=(m == 0), stop=(m == MT - 1), skip_group_check=True)

        if phase == "mlp":
            return
        At, _ = ln_tail(sg, qg, HID, eps3, 1.0, "l3", want_cb=False)
        mrow = lnpool.tile([1, PC2], BF16, tag="mrow")
        nc.vector.tensor_scalar_mul(out=mrow[:, :pc], in0=sg[0:1, :pc],
                                    scalar1=1.0 / float(HID))

        # ---------- w3 + hid-LN scale + residual ----------
        for m in range(KT):
            ps = mmtile()
            for k in range(MT):
                nc.tensor.matmul(ps[:, :pc], lhsT=w3[:, k, m * P:(m + 1) * P],
                                 rhs=g[:, k, :pc], start=(k == 0), stop=False,
                                 skip_group_check=True)
            nc.tensor.matmul(ps[:, :pc], lhsT=w3csr[:, m * P:(m + 1) * P],
                             rhs=mrow[:, :pc], start=False, stop=True,
                             skip_group_check=True)
            yt = ypool.tile([P, PC2], F32, tag="yt")
            nc.vector.tensor_tensor(out=yt[:, :pc], in0=ps[:, :pc],
                                    in1=At[:, :pc], op=OP.mult)
            if w3bc is not None:
                nc.vector.scalar_tensor_tensor(out=yt[:, :pc], in0=yt[:, :pc],
                                               scalar=w3bc[:, m:m + 1],
                                               in1=x1[:, m, :pc], op0=OP.add, op1=OP.add)
            else:
                nc.vector.tensor_tensor(out=yt[:, :pc], in0=yt[:, :pc],
                                        in1=x1[:, m, :pc], op=OP.add)
            nc.sync.dma_start(yT[:, m, c0:c0 + pc], yt[:, :pc])

    def emit_all():
        for wins in pairs:
            emit_pair(wins)
        if phase != "full":
            yt0 = ypool.tile([P, PC2], F32, tag="yt")
            nc.vector.memset(yt0[:], 0.0)
            nc.sync.dma_start(yT[:, 0, 0:PC2], yt0[:])

    if loop_n > 1:
        with tc.For_i(0, loop_n, 1):
            emit_all()
    else:
        emit_all()


def _build(has_biases, nwin_total=NWIN, ncores=N_CORES, loop_n=1, scales=None, phase='full'):
    key = ("prog", USE_SILU, USE_RECIP_FAST, tuple(sorted(has_biases.items())), nwin_total, ncores, loop_n,
           tuple(sorted((scales or {}).items())), phase)
    if key in _cache:
        return _cache[key]
    nc = bacc.Bacc("TRN2", target_bir_lowering=False, debug=False,
                   enable_asserts=False, num_devices=ncores)
    toks = nwin_total * NTOK
    aps = {}
    aps["xT"] = nc.dram_tensor("xT", [DIM, toks], F32, kind="ExternalInput").ap()
    aps["yT"] = nc.dram_tensor("yT", [DIM, toks], F32, kind="ExternalOutput").ap()
    for nm, shp, dt in [("wq", [DIM, DIM], F8E4), ("wk", [DIM, DIM], F8E4),
                        ("wv", [DIM, DIM], F8E4), ("wp", [DIM, DIM], F8E4),
                        ("w1", [DIM, HID], BF16), ("w2", [DIM, HID], BF16),
                        ("w3", [HID, DIM], BF16)]:
        aps[nm] = nc.dram_tensor(nm, shp, dt, kind="ExternalInput").ap()
    aps["cos2"] = nc.dram_tensor("cos2", [P, PC2], BF16, kind="ExternalInput").ap()
    aps["sin2"] = nc.dram_tensor("sin2", [P, PC2], BF16, kind="ExternalInput").ap()
    aps["r2t"] = nc.dram_tensor("r2t", [P, P], BF16, kind="ExternalInput").ap()
    aps["w3csr"] = nc.dram_tensor("w3csr", [1, DIM], BF16, kind="ExternalInput").ap()
    for nm, d in [("qb", DIM), ("kb", DIM), ("vb", DIM), ("pb", DIM),
                  ("w1b", HID), ("w2b", HID)]:
        aps[nm] = (nc.dram_tensor(nm, [1, d], BF16, kind="ExternalInput").ap()
                   if has_biases.get(nm) else None)
    aps["w3b"] = (nc.dram_tensor("w3b", [DIM], F32, kind="ExternalInput").ap()
                  if has_biases.get("w3b") else None)
    sc = scales or {"ds_q": 1.0, "ds_k": 1.0, "ds_v": 1.0, "ds_p": 1.0}
    with tile.TileContext(nc) as tc:
        with ExitStack() as ctx:
            _emit(nc, tc, ctx, aps, has_biases, sc, nwin_total, loop_n, phase)
    nc.compile()
    _cache[key] = nc
    return nc


def _pick_scale(w, target=160.0):
    a = float(np.abs(w).max())
    return 2.0 ** np.floor(np.log2(target / a)) if a > 0 else 1.0


def _host_prep(inputs):
    f = {k: np.asarray(v, np.float32) if hasattr(v, "shape") else v
         for k, v in inputs.items()}
    scale = HD ** -0.5
    wq = f["ln1_w"][:, None] * f["q_w"] * scale
    wk = f["ln1_w"][:, None] * f["k_w"]
    wv = f["ln1_w"][:, None] * f["v_w"]
    qb = (f["ln1_b"] @ f["q_w"] + f["q_b"]) * scale
    kb = f["ln1_b"] @ f["k_w"]
    vb = f["ln1_b"] @ f["v_w"] + f["v_b"]
    wp = f["proj_w"]
    pb = f["proj_b"]
    w1 = f["ln2_w"][:, None] * f["w1_w"]
    w2 = f["ln2_w"][:, None] * f["w2_w"]
    w1b = f["ln2_b"] @ f["w1_w"] + f["w1_b"]
    w2b = f["ln2_b"] @ f["w2_w"] + f["w2_b"]
    w3 = f["ffn_w"][:, None] * f["w3_w"]
    w3b = f["ffn_b"] @ f["w3_w"] + f["w3_b"]

    sq, sk, sv, sp = (_pick_scale(wq), _pick_scale(wk),
                      _pick_scale(wv), _pick_scale(wp))
    scales = {"ds_q": 1.0 / (sq * SA), "ds_k": 1.0 / (sk * SA),
              "ds_v": 1.0 / (sv * SA), "ds_p": 1.0 / (sp * SV)}

    cos, sin = _rope_tables()
    cosT = np.ascontiguousarray(cos.T)
    sinT = np.ascontiguousarray(sin.T)
    cos2 = np.tile(np.concatenate([cosT, cosT], 0), (1, 2))
    sin2 = np.tile(np.concatenate([sinT, sinT], 0), (1, 2))

    r = np.zeros((64, 64), np.float32)
    for i in range(32):
        r[2 * i, 2 * i + 1] = -1.0
        r[2 * i + 1, 2 * i] = 1.0
    r2 = np.zeros((128, 128), np.float32)
    r2[:64, :64] = r
    r2[64:, 64:] = r
    r2t = np.ascontiguousarray(r2.T)

    x = f["x"]
    pad = (-H) % WS
    nw = (H + pad) // WS
    xp = np.pad(x, ((0, 0), (0, pad), (0, pad), (0, 0)))
    t = xp.reshape(B, nw, WS, nw, WS, DIM).transpose(0, 1, 3, 2, 4, 5).reshape(B, NWIN * NTOK, DIM)

    shared = {
        "wq": (wq * sq).astype(F8E4NP), "wk": (wk * sk).astype(F8E4NP),
        "wv": (wv * sv).astype(F8E4NP), "wp": (wp * sp).astype(F8E4NP),
        "w1": w1.astype(BF16NP), "w2": w2.astype(BF16NP), "w3": w3.astype(BF16NP),
        "cos2": cos2.astype(BF16NP), "sin2": sin2.astype(BF16NP),
        "r2t": r2t.astype(BF16NP),
        "w3csr": np.ascontiguousarray(-w3.sum(0)[None, :]).astype(BF16NP),
    }
    # biases pre-scaled into psum units of their group
    brows = {"qb": qb * (sq * SA), "kb": kb * (sk * SA), "vb": vb * (sv * SA),
             "pb": pb * (sp * SV), "w1b": w1b, "w2b": w2b}
    has_biases = {k: bool(np.any(np.asarray(v) != 0.0)) for k, v in brows.items()}
    has_biases["w3b"] = bool(np.any(w3b != 0.0))
    for k, v in brows.items():
        if has_biases[k]:
            shared[k] = np.ascontiguousarray(v[None, :]).astype(BF16NP)
    if has_biases["w3b"]:
        shared["w3b"] = np.ascontiguousarray(w3b, np.float32)

    in_maps = []
    for b in range(B):
        m = dict(shared)
        m["xT"] = np.ascontiguousarray(t[b].T)
        in_maps.append(m)
    return in_maps, has_biases, scales


def _host_post(results):
    pad = (-H) % WS
    nw = (H + pad) // WS
    Hp = H + pad
    y = np.empty((B, H, W, DIM), np.float32)
    for b in range(B):
        yb = np.asarray(results[b]["yT"])
        yw = yb.T.reshape(nw, nw, WS, WS, DIM).transpose(0, 2, 1, 3, 4).reshape(Hp, Hp, DIM)
        y[b] = yw[:H, :W, :]
    return y


def kernel(**inputs):
    in_maps, has_biases, scales = _host_prep(inputs)
    nc = _build(has_biases, scales=scales)
    res = run_bass_kernel_spmd(nc, in_maps, core_ids=list(range(N_CORES)))
    return _host_post(res.results)
